# revision 32
# baseline (speedup 1.0000x reference)
"""Trainium2 Bass kernel for nn_Conv3x3 (3x3 stride-3 switched-capacitor conv).

The 18-step charge-integration recurrence in the reference reduces exactly to

    out[i, j] = S * sum_{a,b} w[a, b] * x[3i+a, 3j+b],   S = -C_RATIO*C_BASE/c2

i.e. a plain 3x3 stride-3 correlation scaled by S ~= -1/0.924458, with the
(1536, 1536) patch grid flattened row-major.

Sharding: the 4608-wide second axis of x is split into 8 column slices of 576
(one per NeuronCore); the weight is replicated.  Each core computes a
(1536, 192) column slice of the patch grid; no cross-core communication.

Per-core kernel: the 1536 patch rows map to 12 full 128-partition tiles
(3 consecutive x rows per partition, loaded with one fully contiguous DMA).
The 9 stencil taps are fused multiply-accumulates on the vector engine
(scalar_tensor_tensor), reading stride-3 slices of the row-triple tile.
"""

import math
import sys

import numpy as np

for _p in ("/opt/trn_rl_repo",):
    if _p not in sys.path:
        sys.path.insert(0, _p)

import concourse.bass as bass
import concourse.mybir as mybir
from concourse.tile import TileContext

# ---- problem constants (hardcoded; must match the reference) ----
N_CORES = 8
W = H = 4608
NW, NH = W // 3, H // 3  # 1536, 1536
COLS = H // N_CORES      # 576 input columns per core
NJ = COLS // 3           # 192 output columns per core

C_BASE = 1e-14
C_RATIO = 100 * (2**4 - 1)  # 1500
INIT_C1_SCALED = 0.924458
_C2 = INIT_C1_SCALED * C_BASE * C_RATIO
SCALE = -(C_BASE / _C2) * C_RATIO  # = -1/INIT_C1_SCALED

# tiling knobs (defaults = current best configuration)
P = 128
KBLOCKS = NW // P            # 12 row-blocks of 128 patch rows
GROUP_SIZES = [1, 2, 3, 3, 2, 1]
LOAD_RINGS = ["sync"] * len(GROUP_SIZES)   # engine issuing each group's load
STORE_RINGS = ["scalar"] * len(GROUP_SIZES)
BUFS = 4


def _legalize_multiwait(nc: bass.Bass) -> int:
    """Walrus codegen in this toolchain accepts at most ONE sync-wait per
    instruction.  Tile can attach several (e.g. the kernel-tail drain waits on
    every semaphore).  Hoist all but the last wait onto standalone
    EventSemaphore no-ops on the same engine, inserted just before the
    instruction — per-engine program order makes this equivalent."""
    n = 0
    for f in nc.m.functions:
        for bb in f.blocks:
            out = []
            for inst in bb.instructions:
                si = inst.sync_info
                if si is not None and si.on_wait and len(si.on_wait) > 1:
                    waits = list(si.on_wait)
                    for j, w in enumerate(waits[:-1]):
                        ev = mybir.InstEventSemaphore(
                            name=f"{inst.name}-hoistw{j}",
                            opcode="EventSemaphore",
                            engine=inst.engine,
                            ins=[],
                            outs=[],
                            sync_info=mybir.SyncInfo(on_wait=[w], on_update=[]),
                        )
                        try:
                            nc.register_instruction(ev, overwrite=True)
                        except Exception:
                            pass
                        out.append(ev)
                        n += 1
                    si.on_wait = [waits[-1]]
                out.append(inst)
            bb.instructions = out
    return n


def build_nc(
    iters: int = 1,
    groups=None,
    load_rings=None,
    store_rings=None,
    bufs: int = None,
    taps: int = 9,          # diagnostic: compute only the first `taps` MACs
    dve_groups=None,        # big-tile mode: decouple DVE grouping from loads
    tt_ops: bool = False,   # diagnostic: taps as plain tensor_add (no scalar)
    psum_acc: bool = False,  # accumulate taps in PSUM, copy to SBUF for store
    weight_imm=None,        # 9 floats (already SCALE-folded): bake as imms
    gp_cols: int = 0,       # j-columns per group computed on GPSIMD (0=off)
    split_load: bool = False,  # load each group as two half-column DMAs
) -> bass.Bass:
    """iters > 1 wraps the whole per-core computation in an on-device For_i
    loop (used only for timing; the graded kernel uses iters=1)."""
    groups = list(GROUP_SIZES if groups is None else groups)
    load_rings = list(LOAD_RINGS if load_rings is None else load_rings)
    store_rings = list(STORE_RINGS if store_rings is None else store_rings)
    bufs = BUFS if bufs is None else bufs
    if dve_groups is not None:
        bufs = 1  # single full-size tile; slices carry the pipelining
    assert sum(groups) == KBLOCKS
    assert len(load_rings) == len(groups)

    nc = bass.Bass()
    x = nc.declare_dram_parameter("x", [W, COLS], mybir.dt.float32, isOutput=False)
    if weight_imm is None:
        wt = nc.declare_dram_parameter(
            "weight", [3, 3], mybir.dt.float32, isOutput=False)
    y = nc.declare_dram_parameter("y", [NW, NJ], mybir.dt.float32, isOutput=True)

    with TileContext(nc) as tc:
        with (
            tc.tile_pool(name="wpool", bufs=1) as wpool,
            tc.tile_pool(name="xpool", bufs=bufs) as xpool,
            tc.tile_pool(name="ypool", bufs=bufs) as ypool,
            tc.tile_pool(name="ppool", bufs=(2 if psum_acc else 1),
                         space="PSUM") as ppool,
        ):
            if weight_imm is None:
                # weight prep on the ACT HWDGE ring so it does not delay the
                # first load on the SP ring.
                wraw = wpool.tile([P, 9], mybir.dt.float32)
                nc.scalar.dma_start(
                    out=wraw[:],
                    in_=wt[:].rearrange("a b -> (a b)")[None, :].to_broadcast((P, 9)),
                )
                wbc = wpool.tile([P, 9], mybir.dt.float32)
                nc.vector.tensor_scalar_mul(wbc[:], wraw[:], float(SCALE))

            def emit_taps(xv, yv, final_out=None, eng=None):
                """yv[...] = sum_k w_k * xv[..., k-tap slice].

                With final_out set, the accumulator is yv (e.g. PSUM) but the
                last tap writes final_out (SBUF) so no extra copy is needed."""
                e = eng if eng is not None else nc.vector
                for k in range(taps):
                    a, b = divmod(k, 3)
                    in0 = xv[:, :, a, b, :]
                    wk = (float(weight_imm[k]) if weight_imm is not None
                          else wbc[:, k:k + 1])
                    out = final_out if (final_out is not None and
                                        k == taps - 1) else yv
                    if k == 0 and eng is not None:
                        # Pool rejects TensorScalarPtr; (in0*w) bypass in1
                        e.scalar_tensor_tensor(
                            out=out, in0=in0, scalar=wk, in1=in0,
                            op0=mybir.AluOpType.mult,
                            op1=mybir.AluOpType.bypass,
                        )
                    elif k == 0:
                        e.tensor_scalar_mul(out, in0, wk)
                    elif tt_ops:
                        e.tensor_add(out, in0, yv)
                    else:
                        e.scalar_tensor_tensor(
                            out=out,
                            in0=in0,
                            scalar=wk,
                            in1=yv,
                            op0=mybir.AluOpType.mult,
                            op1=mybir.AluOpType.add,
                        )

            def body_bigtile():
                """One big x tile; loads land in block slices, DVE tap-groups
                span arbitrary block ranges, stores follow DVE groups."""
                xt = xpool.tile([P, KBLOCKS * 3 * COLS], mybir.dt.float32,
                                name="xtbig", tag="xt")
                yt = ypool.tile([P, KBLOCKS * NJ], mybir.dt.float32,
                                name="ytbig", tag="yt")
                blk = 0
                for g, G in enumerate(groups):
                    r0 = blk * 3 * P
                    rows = G * 3 * P
                    src = x[r0:r0 + rows, :].rearrange(
                        "(g2 p a) c -> p g2 (a c)", g2=G, p=P, a=3
                    )
                    ld_eng = getattr(nc, load_rings[g])
                    ld_eng.dma_start(
                        out=xt[:, blk * 3 * COLS:(blk + G) * 3 * COLS].rearrange(
                            "p (g2 r) -> p g2 r", g2=G),
                        in_=src,
                    )
                    blk += G
                blk = 0
                for d, D in enumerate(dve_groups):
                    xv = xt[:, blk * 3 * COLS:(blk + D) * 3 * COLS].rearrange(
                        "p (g2 a j b) -> p g2 a b j", g2=D, a=3, j=NJ, b=3
                    )
                    yv = yt[:, blk * NJ:(blk + D) * NJ].rearrange(
                        "p (g2 j) -> p g2 j", g2=D)
                    if psum_acc:
                        yp = ppool.tile([P, D * NJ], mybir.dt.float32,
                                        name=f"yp{d}", tag="yp")
                        ypv = yp[:].rearrange("p (g2 j) -> p g2 j", g2=D)
                        emit_taps(xv, ypv, final_out=yv)
                    else:
                        emit_taps(xv, yv)
                    dst = y[blk * P:(blk + D) * P, :].rearrange(
                        "(g2 p) j -> p g2 j", g2=D, p=P
                    )
                    st_eng = getattr(nc, store_rings[d % len(store_rings)])
                    st_eng.dma_start(
                        out=dst,
                        in_=yt[:, blk * NJ:(blk + D) * NJ].rearrange(
                            "p (g2 j) -> p g2 j", g2=D),
                    )
                    blk += D

            def body():
                blk = 0  # first k-block of the current group
                for g, G in enumerate(groups):
                    r0 = blk * 3 * P          # first x row of the group
                    rows = G * 3 * P
                    xt = xpool.tile([P, G * 3 * COLS], mybir.dt.float32,
                                    name=f"xt{g}", tag="xt")
                    if split_load:
                        half = COLS // 2
                        xtv = xt[:].rearrange(
                            "p (g2 a c) -> p g2 a c", g2=G, a=3)
                        for g2 in range(G):
                            rr = r0 + g2 * 3 * P
                            for h, eng in ((0, nc.sync), (1, nc.scalar)):
                                c0 = h * half
                                srch = x[rr:rr + 3 * P, c0:c0 + half].rearrange(
                                    "(p a) c -> p a c", p=P, a=3
                                )
                                eng.dma_start(
                                    out=xtv[:, g2, :, c0:c0 + half], in_=srch
                                )
                    else:
                        ld_eng = getattr(nc, load_rings[g])
                        src = x[r0:r0 + rows, :].rearrange(
                            "(g2 p a) c -> p g2 (a c)", g2=G, p=P, a=3
                        )
                        ld_eng.dma_start(
                            out=xt[:].rearrange("p (g2 r) -> p g2 r", g2=G),
                            in_=src,
                        )
                    # [p][g2][a][b][j] view of the row-triple tile
                    xv = xt[:].rearrange(
                        "p (g2 a j b) -> p g2 a b j", g2=G, a=3, j=NJ, b=3
                    )
                    yt = ypool.tile([P, G * NJ], mybir.dt.float32,
                                    name=f"yt{g}", tag="yt")
                    yv = yt[:].rearrange("p (g2 j) -> p g2 j", g2=G)
                    ndj = NJ - gp_cols  # j-columns handled on DVE
                    if psum_acc:
                        yp = ppool.tile([P, G * NJ], mybir.dt.float32,
                                        name=f"yp{g}", tag="yp")
                        ypv = yp[:].rearrange("p (g2 j) -> p g2 j", g2=G)
                        emit_taps(xv, ypv, final_out=yv)
                    elif gp_cols > 0:
                        # DVE covers j < ndj; GPSIMD covers the rest.
                        emit_taps(xv[:, :, :, :, 0:ndj], yv[:, :, 0:ndj])
                        emit_taps(xv[:, :, :, :, ndj:NJ], yv[:, :, ndj:NJ],
                                  eng=nc.gpsimd)
                    else:
                        emit_taps(xv, yv)
                    dst = y[blk * P:(blk + G) * P, :].rearrange(
                        "(g2 p) j -> p g2 j", g2=G, p=P
                    )
                    st_eng = getattr(nc, store_rings[g])
                    st_eng.dma_start(out=dst, in_=yt[:].rearrange(
                        "p (g2 j) -> p g2 j", g2=G
                    ))
                    blk += G

            the_body = body if dve_groups is None else body_bigtile
            if dve_groups is not None:
                assert sum(dve_groups) == KBLOCKS
            if iters == 1:
                the_body()
            else:
                with tc.For_i(0, iters, 1):
                    the_body()
    _legalize_multiwait(nc)
    return nc


_CACHED = {}

# graded configuration (best measured variant):
# groups [1,2,3,3,2,1], loads on the SP HWDGE ring, stores on ACT, bufs=4,
# and the 3x3 weights (folded with the -1/0.924458 readout scale) baked into
# the tap instructions as immediates — drops the weight DMA + broadcast from
# the critical path. Compilation is keyed on the weight bytes, so a new
# weight value just triggers one (cached) recompile on the host.
BEST_CFG = {}
USE_WEIGHT_IMM = True


def _get_nc(weight: np.ndarray) -> bass.Bass:
    if USE_WEIGHT_IMM:
        wimm = tuple(float(v) * SCALE for v in np.asarray(weight).reshape(9))
        key = ("imm", wimm)
        if key not in _CACHED:
            _CACHED[key] = build_nc(weight_imm=list(wimm), **BEST_CFG)
        return _CACHED[key]
    if "nc" not in _CACHED:
        _CACHED["nc"] = build_nc(**BEST_CFG)
    return _CACHED["nc"]


def _run_pjrt_nodonate(nc: bass.Bass, in_maps, n_cores: int):
    """Minimal SPMD PJRT runner mirroring bass2jax.run_bass_via_pjrt but
    WITHOUT output-buffer donation (the donated-zero-outputs path raises an
    INTERNAL error on some axon client builds; our kernel writes every
    output element, so uninitialized/zero output operands are equivalent)."""
    import jax
    from jax.sharding import Mesh, PartitionSpec
    try:
        from jax import shard_map
    except ImportError:
        from jax.experimental.shard_map import shard_map
    from concourse import bass2jax

    bass2jax.install_neuronx_cc_hook()
    partition_name = (
        nc.partition_id_tensor.name if nc.partition_id_tensor else None)

    in_names, out_names, out_avals, zero_outs = [], [], [], []
    for alloc in nc.m.functions[0].allocations:
        if not isinstance(alloc, mybir.MemoryLocationSet):
            continue
        name = alloc.memorylocations[0].name
        if alloc.kind == "ExternalInput":
            if name != partition_name:
                in_names.append(name)
        elif alloc.kind == "ExternalOutput":
            shape = tuple(alloc.tensor_shape)
            dtype = mybir.dt.np(alloc.dtype)
            out_avals.append(jax.core.ShapedArray(shape, dtype))
            out_names.append(name)
            zero_outs.append(np.zeros(shape, dtype))
    n_params = len(in_names)
    all_names = in_names + out_names
    if partition_name is not None:
        all_names.append(partition_name)

    def _body(*args):
        operands = list(args)
        if partition_name is not None:
            operands.append(bass2jax.partition_id_tensor())
        outs = bass2jax._bass_exec_p.bind(
            *operands,
            out_avals=tuple(out_avals),
            in_names=tuple(all_names),
            out_names=tuple(out_names),
            lowering_input_output_aliases=(),
            sim_require_finite=True,
            sim_require_nnan=True,
            nc=nc,
        )
        return tuple(outs)

    devices = jax.devices()[:n_cores]
    mesh = Mesh(np.asarray(devices), ("core",))
    sharded = jax.jit(
        shard_map(
            _body,
            mesh=mesh,
            in_specs=(PartitionSpec("core"),) * (n_params + len(out_avals)),
            out_specs=(PartitionSpec("core"),) * len(out_avals),
            check_rep=False,
        ),
        keep_unused=True,
    )
    concat_in = [
        np.concatenate([np.asarray(in_maps[c][nm]) for c in range(n_cores)], axis=0)
        for nm in in_names
    ]
    concat_zeros = [
        np.zeros((n_cores * z.shape[0], *z.shape[1:]), z.dtype) for z in zero_outs
    ]
    out_arrs = sharded(*concat_in, *concat_zeros)
    return [
        {
            nm: np.asarray(out_arrs[i]).reshape(n_cores, *out_avals[i].shape)[c]
            for i, nm in enumerate(out_names)
        }
        for c in range(n_cores)
    ]


def kernel(**inputs: np.ndarray) -> np.ndarray:
    from concourse import bass_utils

    x = np.ascontiguousarray(np.asarray(inputs["x"], dtype=np.float32))
    weight = np.ascontiguousarray(np.asarray(inputs["weight"], dtype=np.float32))
    assert x.shape == (W, H) and weight.shape == (3, 3)

    nc = _get_nc(weight)
    in_maps = [
        {
            "x": np.ascontiguousarray(x[:, m * COLS:(m + 1) * COLS]),
            **({} if USE_WEIGHT_IMM else {"weight": weight}),
        }
        for m in range(N_CORES)
    ]
    try:
        res = bass_utils.run_bass_kernel_spmd(
            nc, in_maps, core_ids=list(range(N_CORES)))
        results = res.results
    except Exception:
        # donation-based PJRT path can fail on some axon clients; retry
        # with the no-donation runner (identical math, fresh zero outputs)
        results = _run_pjrt_nodonate(nc, in_maps, N_CORES)
    out2d = np.empty((NW, NH), dtype=np.float32)
    for m in range(N_CORES):
        out2d[:, m * NJ:(m + 1) * NJ] = results[m]["y"]
    return out2d.reshape(-1)


# revision 38
# speedup vs baseline: 1.0086x; 1.0086x over previous
"""Trainium2 Bass kernel for nn_Conv3x3 (3x3 stride-3 switched-capacitor conv).

The 18-step charge-integration recurrence in the reference reduces exactly to

    out[i, j] = S * sum_{a,b} w[a, b] * x[3i+a, 3j+b],   S = -C_RATIO*C_BASE/c2

i.e. a plain 3x3 stride-3 correlation scaled by S ~= -1/0.924458, with the
(1536, 1536) patch grid flattened row-major.

Sharding: the 4608-wide second axis of x is split into 8 column slices of 576
(one per NeuronCore); the weight is replicated.  Each core computes a
(1536, 192) column slice of the patch grid; no cross-core communication.

Per-core kernel: the 1536 patch rows map to 12 full 128-partition row-blocks
(3 consecutive x rows per partition, each block loaded with one fully
contiguous DMA on the SP HWDGE ring; stores ride the ACT ring).  Blocks are
loaded in groups of [1,2,3,3,2,1] — small first group so the vector engine
starts early, small last group to minimise the drain tail; measured faster
than coarser or finer schedules.  The 9 stencil taps are fused
multiply-accumulates on the vector engine (scalar_tensor_tensor) reading
stride-3 slices of the row-triple tile, with the weightxscale values baked
into the instructions as immediates (compilation is cached per weight bytes),
which removes the weight load/broadcast from the critical path.
"""

import math
import sys

import numpy as np

for _p in ("/opt/trn_rl_repo",):
    if _p not in sys.path:
        sys.path.insert(0, _p)

import concourse.bass as bass
import concourse.mybir as mybir
from concourse.tile import TileContext

# ---- problem constants (hardcoded; must match the reference) ----
N_CORES = 8
W = H = 4608
NW, NH = W // 3, H // 3  # 1536, 1536
COLS = H // N_CORES      # 576 input columns per core
NJ = COLS // 3           # 192 output columns per core

C_BASE = 1e-14
C_RATIO = 100 * (2**4 - 1)  # 1500
INIT_C1_SCALED = 0.924458
_C2 = INIT_C1_SCALED * C_BASE * C_RATIO
SCALE = -(C_BASE / _C2) * C_RATIO  # = -1/INIT_C1_SCALED

# tiling knobs (defaults = current best configuration)
P = 128
KBLOCKS = NW // P            # 12 row-blocks of 128 patch rows
GROUP_SIZES = [1, 2, 3, 3, 2, 1]
LOAD_RINGS = ["sync"] * len(GROUP_SIZES)   # engine issuing each group's load
STORE_RINGS = ["scalar"] * len(GROUP_SIZES)
BUFS = 4


def _legalize_multiwait(nc: bass.Bass) -> int:
    """Walrus codegen in this toolchain accepts at most ONE sync-wait per
    instruction.  Tile can attach several (e.g. the kernel-tail drain waits on
    every semaphore).  Hoist all but the last wait onto standalone
    EventSemaphore no-ops on the same engine, inserted just before the
    instruction — per-engine program order makes this equivalent."""
    n = 0
    for f in nc.m.functions:
        for bb in f.blocks:
            out = []
            for inst in bb.instructions:
                si = inst.sync_info
                if si is not None and si.on_wait and len(si.on_wait) > 1:
                    waits = list(si.on_wait)
                    for j, w in enumerate(waits[:-1]):
                        ev = mybir.InstEventSemaphore(
                            name=f"{inst.name}-hoistw{j}",
                            opcode="EventSemaphore",
                            engine=inst.engine,
                            ins=[],
                            outs=[],
                            sync_info=mybir.SyncInfo(on_wait=[w], on_update=[]),
                        )
                        try:
                            nc.register_instruction(ev, overwrite=True)
                        except Exception:
                            pass
                        out.append(ev)
                        n += 1
                    si.on_wait = [waits[-1]]
                out.append(inst)
            bb.instructions = out
    return n


def build_nc(
    iters: int = 1,
    groups=None,
    load_rings=None,
    store_rings=None,
    bufs: int = None,
    taps: int = 9,          # diagnostic: compute only the first `taps` MACs
    dve_groups=None,        # big-tile mode: decouple DVE grouping from loads
    tt_ops: bool = False,   # diagnostic: taps as plain tensor_add (no scalar)
    psum_acc: bool = False,  # accumulate taps in PSUM, copy to SBUF for store
    weight_imm=None,        # 9 floats (already SCALE-folded): bake as imms
    gp_cols: int = 0,       # j-columns per group computed on GPSIMD (0=off)
    split_load: bool = False,  # load each group as two half-column DMAs
    chains: int = 1,        # independent interleaved tap chains per group
) -> bass.Bass:
    """iters > 1 wraps the whole per-core computation in an on-device For_i
    loop (used only for timing; the graded kernel uses iters=1)."""
    groups = list(GROUP_SIZES if groups is None else groups)
    load_rings = list(LOAD_RINGS if load_rings is None else load_rings)
    store_rings = list(STORE_RINGS if store_rings is None else store_rings)
    bufs = BUFS if bufs is None else bufs
    if dve_groups is not None:
        bufs = 1  # single full-size tile; slices carry the pipelining
    assert sum(groups) == KBLOCKS
    assert len(load_rings) == len(groups)

    nc = bass.Bass()
    x = nc.declare_dram_parameter("x", [W, COLS], mybir.dt.float32, isOutput=False)
    if weight_imm is None:
        wt = nc.declare_dram_parameter(
            "weight", [3, 3], mybir.dt.float32, isOutput=False)
    y = nc.declare_dram_parameter("y", [NW, NJ], mybir.dt.float32, isOutput=True)

    with TileContext(nc) as tc:
        with (
            tc.tile_pool(name="wpool", bufs=1) as wpool,
            tc.tile_pool(name="xpool", bufs=bufs) as xpool,
            tc.tile_pool(name="ypool", bufs=bufs) as ypool,
            tc.tile_pool(name="ppool", bufs=(2 if psum_acc else 1),
                         space="PSUM") as ppool,
        ):
            if weight_imm is None:
                # weight prep on the ACT HWDGE ring so it does not delay the
                # first load on the SP ring.
                wraw = wpool.tile([P, 9], mybir.dt.float32)
                nc.scalar.dma_start(
                    out=wraw[:],
                    in_=wt[:].rearrange("a b -> (a b)")[None, :].to_broadcast((P, 9)),
                )
                wbc = wpool.tile([P, 9], mybir.dt.float32)
                nc.vector.tensor_scalar_mul(wbc[:], wraw[:], float(SCALE))

            def emit_taps(xv, yv, final_out=None, eng=None):
                """yv[...] = sum_k w_k * xv[..., k-tap slice].

                With final_out set, the accumulator is yv (e.g. PSUM) but the
                last tap writes final_out (SBUF) so no extra copy is needed.

                With chains > 1 the j-range is split into `chains` slices with
                independent accumulation chains, ops interleaved in program
                order: Tile gates each DVE op on the previous op of ITS chain
                (completion-event semaphore), so alternating chains hides the
                completion/sem latency under the other chain's streaming."""
                e = eng if eng is not None else nc.vector
                nj = yv.shape[-1]
                cuts = [(nj * c) // chains for c in range(chains + 1)]
                for k in range(taps):
                    a, b = divmod(k, 3)
                    wk = (float(weight_imm[k]) if weight_imm is not None
                          else wbc[:, k:k + 1])
                    for c in range(chains):
                        j0, j1 = cuts[c], cuts[c + 1]
                        in0 = xv[:, :, a, b, j0:j1]
                        outv = (final_out if (final_out is not None and
                                              k == taps - 1) else yv)
                        out = outv[:, :, j0:j1]
                        acc = yv[:, :, j0:j1]
                        if k == 0 and eng is not None:
                            # Pool rejects TensorScalarPtr; (in0*w) bypass in1
                            e.scalar_tensor_tensor(
                                out=out, in0=in0, scalar=wk, in1=in0,
                                op0=mybir.AluOpType.mult,
                                op1=mybir.AluOpType.bypass,
                            )
                        elif k == 0:
                            e.tensor_scalar_mul(out, in0, wk)
                        elif tt_ops:
                            e.tensor_add(out, in0, acc)
                        else:
                            e.scalar_tensor_tensor(
                                out=out,
                                in0=in0,
                                scalar=wk,
                                in1=acc,
                                op0=mybir.AluOpType.mult,
                                op1=mybir.AluOpType.add,
                            )

            def body_bigtile():
                """One big x tile; loads land in block slices, DVE tap-groups
                span arbitrary block ranges, stores follow DVE groups."""
                xt = xpool.tile([P, KBLOCKS * 3 * COLS], mybir.dt.float32,
                                name="xtbig", tag="xt")
                yt = ypool.tile([P, KBLOCKS * NJ], mybir.dt.float32,
                                name="ytbig", tag="yt")
                blk = 0
                for g, G in enumerate(groups):
                    r0 = blk * 3 * P
                    rows = G * 3 * P
                    src = x[r0:r0 + rows, :].rearrange(
                        "(g2 p a) c -> p g2 (a c)", g2=G, p=P, a=3
                    )
                    ld_eng = getattr(nc, load_rings[g])
                    ld_eng.dma_start(
                        out=xt[:, blk * 3 * COLS:(blk + G) * 3 * COLS].rearrange(
                            "p (g2 r) -> p g2 r", g2=G),
                        in_=src,
                    )
                    blk += G
                blk = 0
                for d, D in enumerate(dve_groups):
                    xv = xt[:, blk * 3 * COLS:(blk + D) * 3 * COLS].rearrange(
                        "p (g2 a j b) -> p g2 a b j", g2=D, a=3, j=NJ, b=3
                    )
                    yv = yt[:, blk * NJ:(blk + D) * NJ].rearrange(
                        "p (g2 j) -> p g2 j", g2=D)
                    if psum_acc:
                        yp = ppool.tile([P, D * NJ], mybir.dt.float32,
                                        name=f"yp{d}", tag="yp")
                        ypv = yp[:].rearrange("p (g2 j) -> p g2 j", g2=D)
                        emit_taps(xv, ypv, final_out=yv)
                    else:
                        emit_taps(xv, yv)
                    dst = y[blk * P:(blk + D) * P, :].rearrange(
                        "(g2 p) j -> p g2 j", g2=D, p=P
                    )
                    st_eng = getattr(nc, store_rings[d % len(store_rings)])
                    st_eng.dma_start(
                        out=dst,
                        in_=yt[:, blk * NJ:(blk + D) * NJ].rearrange(
                            "p (g2 j) -> p g2 j", g2=D),
                    )
                    blk += D

            def body():
                blk = 0  # first k-block of the current group
                for g, G in enumerate(groups):
                    r0 = blk * 3 * P          # first x row of the group
                    rows = G * 3 * P
                    xt = xpool.tile([P, G * 3 * COLS], mybir.dt.float32,
                                    name=f"xt{g}", tag="xt")
                    if split_load:
                        half = COLS // 2
                        xtv = xt[:].rearrange(
                            "p (g2 a c) -> p g2 a c", g2=G, a=3)
                        for g2 in range(G):
                            rr = r0 + g2 * 3 * P
                            for h, eng in ((0, nc.sync), (1, nc.scalar)):
                                c0 = h * half
                                srch = x[rr:rr + 3 * P, c0:c0 + half].rearrange(
                                    "(p a) c -> p a c", p=P, a=3
                                )
                                eng.dma_start(
                                    out=xtv[:, g2, :, c0:c0 + half], in_=srch
                                )
                    else:
                        ld_eng = getattr(nc, load_rings[g])
                        src = x[r0:r0 + rows, :].rearrange(
                            "(g2 p a) c -> p g2 (a c)", g2=G, p=P, a=3
                        )
                        ld_eng.dma_start(
                            out=xt[:].rearrange("p (g2 r) -> p g2 r", g2=G),
                            in_=src,
                        )
                    # [p][g2][a][b][j] view of the row-triple tile
                    xv = xt[:].rearrange(
                        "p (g2 a j b) -> p g2 a b j", g2=G, a=3, j=NJ, b=3
                    )
                    yt = ypool.tile([P, G * NJ], mybir.dt.float32,
                                    name=f"yt{g}", tag="yt")
                    yv = yt[:].rearrange("p (g2 j) -> p g2 j", g2=G)
                    ndj = NJ - gp_cols  # j-columns handled on DVE
                    if psum_acc:
                        yp = ppool.tile([P, G * NJ], mybir.dt.float32,
                                        name=f"yp{g}", tag="yp")
                        ypv = yp[:].rearrange("p (g2 j) -> p g2 j", g2=G)
                        emit_taps(xv, ypv, final_out=yv)
                    elif gp_cols > 0:
                        # DVE covers j < ndj; GPSIMD covers the rest.
                        emit_taps(xv[:, :, :, :, 0:ndj], yv[:, :, 0:ndj])
                        emit_taps(xv[:, :, :, :, ndj:NJ], yv[:, :, ndj:NJ],
                                  eng=nc.gpsimd)
                    else:
                        emit_taps(xv, yv)
                    dst = y[blk * P:(blk + G) * P, :].rearrange(
                        "(g2 p) j -> p g2 j", g2=G, p=P
                    )
                    st_eng = getattr(nc, store_rings[g])
                    st_eng.dma_start(out=dst, in_=yt[:].rearrange(
                        "p (g2 j) -> p g2 j", g2=G
                    ))
                    blk += G

            the_body = body if dve_groups is None else body_bigtile
            if dve_groups is not None:
                assert sum(dve_groups) == KBLOCKS
            if iters == 1:
                the_body()
            else:
                with tc.For_i(0, iters, 1):
                    the_body()
    _legalize_multiwait(nc)
    return nc


_CACHED = {}

# graded configuration (best measured variant):
# groups [1,2,3,3,2,1], loads on the SP HWDGE ring, stores on ACT, bufs=4,
# and the 3x3 weights (folded with the -1/0.924458 readout scale) baked into
# the tap instructions as immediates — drops the weight DMA + broadcast from
# the critical path. Compilation is keyed on the weight bytes, so a new
# weight value just triggers one (cached) recompile on the host.
BEST_CFG = {"chains": 2}
USE_WEIGHT_IMM = True


def _get_nc(weight: np.ndarray) -> bass.Bass:
    if USE_WEIGHT_IMM:
        wimm = tuple(float(v) * SCALE for v in np.asarray(weight).reshape(9))
        key = ("imm", wimm)
        if key not in _CACHED:
            _CACHED[key] = build_nc(weight_imm=list(wimm), **BEST_CFG)
        return _CACHED[key]
    if "nc" not in _CACHED:
        _CACHED["nc"] = build_nc(**BEST_CFG)
    return _CACHED["nc"]


def _run_pjrt_nodonate(nc: bass.Bass, in_maps, n_cores: int):
    """Minimal SPMD PJRT runner mirroring bass2jax.run_bass_via_pjrt but
    WITHOUT output-buffer donation (the donated-zero-outputs path raises an
    INTERNAL error on some axon client builds; our kernel writes every
    output element, so uninitialized/zero output operands are equivalent)."""
    import jax
    from jax.sharding import Mesh, PartitionSpec
    try:
        from jax import shard_map
    except ImportError:
        from jax.experimental.shard_map import shard_map
    from concourse import bass2jax

    bass2jax.install_neuronx_cc_hook()
    partition_name = (
        nc.partition_id_tensor.name if nc.partition_id_tensor else None)

    in_names, out_names, out_avals, zero_outs = [], [], [], []
    for alloc in nc.m.functions[0].allocations:
        if not isinstance(alloc, mybir.MemoryLocationSet):
            continue
        name = alloc.memorylocations[0].name
        if alloc.kind == "ExternalInput":
            if name != partition_name:
                in_names.append(name)
        elif alloc.kind == "ExternalOutput":
            shape = tuple(alloc.tensor_shape)
            dtype = mybir.dt.np(alloc.dtype)
            out_avals.append(jax.core.ShapedArray(shape, dtype))
            out_names.append(name)
            zero_outs.append(np.zeros(shape, dtype))
    n_params = len(in_names)
    all_names = in_names + out_names
    if partition_name is not None:
        all_names.append(partition_name)

    def _body(*args):
        operands = list(args)
        if partition_name is not None:
            operands.append(bass2jax.partition_id_tensor())
        outs = bass2jax._bass_exec_p.bind(
            *operands,
            out_avals=tuple(out_avals),
            in_names=tuple(all_names),
            out_names=tuple(out_names),
            lowering_input_output_aliases=(),
            sim_require_finite=True,
            sim_require_nnan=True,
            nc=nc,
        )
        return tuple(outs)

    devices = jax.devices()[:n_cores]
    mesh = Mesh(np.asarray(devices), ("core",))
    sharded = jax.jit(
        shard_map(
            _body,
            mesh=mesh,
            in_specs=(PartitionSpec("core"),) * (n_params + len(out_avals)),
            out_specs=(PartitionSpec("core"),) * len(out_avals),
            check_rep=False,
        ),
        keep_unused=True,
    )
    concat_in = [
        np.concatenate([np.asarray(in_maps[c][nm]) for c in range(n_cores)], axis=0)
        for nm in in_names
    ]
    concat_zeros = [
        np.zeros((n_cores * z.shape[0], *z.shape[1:]), z.dtype) for z in zero_outs
    ]
    out_arrs = sharded(*concat_in, *concat_zeros)
    return [
        {
            nm: np.asarray(out_arrs[i]).reshape(n_cores, *out_avals[i].shape)[c]
            for i, nm in enumerate(out_names)
        }
        for c in range(n_cores)
    ]


def kernel(**inputs: np.ndarray) -> np.ndarray:
    from concourse import bass_utils

    x = np.ascontiguousarray(np.asarray(inputs["x"], dtype=np.float32))
    weight = np.ascontiguousarray(np.asarray(inputs["weight"], dtype=np.float32))
    assert x.shape == (W, H) and weight.shape == (3, 3)

    nc = _get_nc(weight)
    in_maps = [
        {
            "x": np.ascontiguousarray(x[:, m * COLS:(m + 1) * COLS]),
            **({} if USE_WEIGHT_IMM else {"weight": weight}),
        }
        for m in range(N_CORES)
    ]
    try:
        # run_bass_kernel_spmd's PJRT path donates zero output buffers, which
        # raises INTERNAL on some axon clients; the no-donation runner is
        # semantically identical (every output element is written), so prefer
        # it and keep the stock path as fallback.
        results = _run_pjrt_nodonate(nc, in_maps, N_CORES)
    except Exception:
        res = bass_utils.run_bass_kernel_spmd(
            nc, in_maps, core_ids=list(range(N_CORES)))
        results = res.results
    out2d = np.empty((NW, NH), dtype=np.float32)
    for m in range(N_CORES):
        out2d[:, m * NJ:(m + 1) * NJ] = results[m]["y"]
    return out2d.reshape(-1)
